# revision 40
# baseline (speedup 1.0000x reference)
"""Trainium2 Bass kernel for the Encoder-z0 ODE-ConvGRU problem.

Data-parallel over batch: 16 batch elements / 8 NeuronCores = 2 per core.
Per core, a 16-step backwards ConvGRU recurrence with an Euler ODE step,
followed by a 1x1-conv transform producing (mean_z0, std_z0).

Conv3x3 (SAME) is computed as 9 shifted matmuls accumulating in PSUM:
feature maps live in SBUF as zero-padded (34x34) images with channels on
partitions; offset (dy,dx) contributes lhsT[k].T @ shifted_view(rhs).

The two local batch elements are laid out on opposite partition halves
(b=0: 0-63, b=1: 64-127).  All M=64 convolutions (ODE, candidate halves,
first 1x1) are merged across the two batch elements into single full-array
K=128 x M=128 matmuls with block-diagonal weights, halving their PE time.
The candidate conv splits into an x-part (independent of the recurrent
state) whose two 9-matmul halves are scheduled to plug the recurrence's
two serial stalls, and an rh-part accumulating into the same PSUM banks.

Each conv output-row half gets its own single-bank PSUM tile and its own
dense SBUF activation tile, so Tile's (tile-granular) dependency tracking
yields precise chains: activations start as soon as their bank's
accumulation group stops, and the h_next/h_ode elementwise chains expose
only ~1-2us per step, hidden behind filler matmuls.

dt = -1 steps fold the Euler scale into negated ODE weights (tanh is odd).
Matmuls run in float32r (full-rate PE streaming, ~1e-3 accuracy);
elementwise runs in fp32.
"""

import os

import numpy as np

import concourse.bass as bass
import concourse.tile as tile
from concourse import bacc, mybir
from concourse import bass_utils

B, T, C, H, W = 16, 16, 64, 32, 32
HD = 64
NCORES = 8
BL = B // NCORES          # batch elements per core
P = H + 2                 # padded image edge (34)
NPIX = H * W              # 1024
MMD = mybir.dt.float32r   # matmul dtype
F32 = mybir.dt.float32

last_result = None


def _offsets():
    return [(dy, dx) for dy in range(3) for dx in range(3)]


def _build(dts, use_mask, t0):
    nc = bacc.Bacc("TRN2", target_bir_lowering=False, debug=False,
                   num_devices=NCORES)

    FC = 2 * C  # 128
    TD = T - t0  # device steps
    xs_d = nc.dram_tensor("xs", [TD, FC, P, P], MMD, kind="ExternalInput").ap()
    h0_d = nc.dram_tensor("h0", [FC, P, P], MMD, kind="ExternalInput").ap()
    wg_d = nc.dram_tensor("wg", [BL, FC, 9 * FC], MMD, kind="ExternalInput").ap()
    need_plain = any(float(dt) != -1.0 for dt in dts)
    need_neg = any(float(dt) == -1.0 for dt in dts)
    nv = int(need_plain) + int(need_neg)
    wcx_d = nc.dram_tensor("wcx", [C, 9 * C], MMD, kind="ExternalInput").ap()
    wch_d = nc.dram_tensor("wch", [C, 9 * C], MMD, kind="ExternalInput").ap()
    wo_d = nc.dram_tensor("wo", [C, nv * 9 * C], MMD, kind="ExternalInput").ap()
    wt1_d = nc.dram_tensor("wt1", [FC, FC], MMD, kind="ExternalInput").ap()
    wt2_d = nc.dram_tensor("wt2", [FC, FC], MMD, kind="ExternalInput").ap()
    bg_d = nc.dram_tensor("bg", [BL, FC, 1], F32, kind="ExternalInput").ap()
    bc_d = nc.dram_tensor("bc", [FC, 1], F32, kind="ExternalInput").ap()
    bo_d = nc.dram_tensor("bo", [FC, 2], F32, kind="ExternalInput").ap()
    bt1_d = nc.dram_tensor("bt1", [FC, 1], F32, kind="ExternalInput").ap()
    bt2_d = nc.dram_tensor("bt2", [FC, 1], F32, kind="ExternalInput").ap()
    if use_mask:
        msd = nc.dram_tensor("ms", [TD, BL, HD, 1], F32, kind="ExternalInput").ap()
    mean_d = nc.dram_tensor("mean", [BL, HD, H, W], F32, kind="ExternalOutput").ap()
    std_d = nc.dram_tensor("std", [BL, HD, H, W], F32, kind="ExternalOutput").ap()

    AF = mybir.ActivationFunctionType
    offs = _offsets()

    with tile.TileContext(nc) as tc:
        with (
            tc.tile_pool(name="persist", bufs=1) as pp,
            tc.tile_pool(name="ew", bufs=3) as ew,
            tc.tile_pool(name="psum", bufs=8, space="PSUM") as psp,
        ):
            # ---- persistent state ----
            hbuf = pp.tile([FC, P, P], MMD, name="hbuf")    # h: b0 low, b1 high
            xbuf = [pp.tile([FC, P, P], MMD, name=f"xbuf{i}")  # double-buffered
                    for i in range(2)]
            rhbuf = pp.tile([FC, P, P], MMD, name="rhbuf")  # r*h_ode per half
            bufa = [pp.tile([FC, P, P], MMD, name=f"bufa{b}") for b in range(BL)]
            wg = [pp.tile([FC, 9 * FC], MMD, name=f"wg{b}") for b in range(BL)]
            wcx = pp.tile([FC, 9 * FC], MMD, name="wcx")
            wch = pp.tile([FC, 9 * FC], MMD, name="wch")
            wo = pp.tile([FC, nv * 9 * FC], MMD, name="wo")
            wt1 = pp.tile([FC, FC], MMD, name="wt1")
            wt2 = pp.tile([FC, FC], MMD, name="wt2")
            bg = [pp.tile([FC, 1], F32, name=f"bg{b}") for b in range(BL)]
            bc = pp.tile([FC, 1], F32, name="bc")
            bo = pp.tile([FC, 2], F32, name="bo")           # [plain, negated]
            bt1 = pp.tile([FC, 1], F32, name="bt1")
            bt2 = pp.tile([FC, 1], F32, name="bt2")

            # zero only the pad borders (interiors are written before use)
            engs = [nc.vector, nc.gpsimd]
            for i, buf in enumerate([rhbuf, bufa[0], bufa[1]]):
                e = engs[i % 2]
                e.memzero(buf[:, 0, :])
                e.memzero(buf[:, 33, :])
                e.memzero(buf[:, 1:33, 0:1])
                e.memzero(buf[:, 1:33, 33:34])
            def expand_diag(wtile, src, nq, eng):
                """Fill the two diagonal 64x64 blocks of each 128-col chunk
                from a dense (64, nq*64) DRAM tensor (upper half zeroed by
                memzero beforehand)."""
                eng.memzero(wtile[:])
                srcv = src.rearrange("p (q m) -> p q m", m=C)
                for hh in range(2):
                    dst = wtile[C * hh:C * hh + C, :].rearrange(
                        "p (q m) -> p q m", m=FC)[:, :, C * hh:C * hh + C]
                    (nc.sync if hh == 0 else nc.scalar).dma_start(dst, srcv)

            # staged init: the first conv blocks wait only on their own
            # transfers; big loads are split across queues for bandwidth
            for q in range(4):
                eng = nc.sync if q % 2 == 0 else nc.scalar
                eng.dma_start(xbuf[0][32 * q:32 * (q + 1), :, :],
                              xs_d[0, 32 * q:32 * (q + 1)])
            expand_diag(wcx, wcx_d[:], 9, nc.vector)

            def convj(psum_t, wtile, wcol0, rhs_buf, j, first, last):
                """Conv matmuls for output-row half j into a 1-bank tile."""
                r0 = 16 * j
                for k, (dy, dx) in enumerate(offs):
                    nc.tensor.matmul(
                        psum_t[:],
                        wtile[:, wcol0 + FC * k:wcol0 + FC * (k + 1)],
                        rhs_buf[:, dy + r0:dy + r0 + 16, dx:dx + 32],
                        start=(first and k == 0), stop=(last and k == 8),
                        skip_group_check=True,
                    )

            def intr(buf, p0, pn):
                return buf[p0:p0 + pn, 1:33, 1:33]

            def intrr(buf, p0, pn, r0, rn):
                return buf[p0:p0 + pn, 1 + r0:1 + r0 + rn, 1:33]

            def r3c(ap):  # dense 512-col chunk -> (p, 16, 32)
                return ap.rearrange("p (y x) -> p y x", y=16, x=W)

            def r3(ap):
                return ap.rearrange("p (y x) -> p y x", y=H, x=W)

            def load_x(t):
                nc.sync.dma_start(xbuf[t % 2][:], xs_d[t])
                nc.sync.dma_start(bufa[0][C:FC, :, :], xs_d[t, 0:C])
                nc.sync.dma_start(bufa[1][0:C, :, :], xs_d[t, C:FC])

            load_x(0)

            for t in range(TD):
                if dts[t] == -1.0:
                    wcol = 9 * FC * int(need_plain)
                    neg = 1
                else:
                    wcol = 0
                    neg = 0

                # canx j0: fills the previous step's h_next tail
                pc = [psp.tile([FC, 512], F32, tag="ps", name=f"pc{j}")
                      for j in range(2)]
                convj(pc[0], wcx, 0, xbuf[t % 2], 0, True, False)

                if t == 0:
                    for q in range(4):
                        eng = nc.sync if q % 2 == 0 else nc.scalar
                        eng.dma_start(hbuf[32 * q:32 * (q + 1), :, :],
                                      h0_d[32 * q:32 * (q + 1)])
                    expand_diag(wo, wo_d[:], nv * 9, nc.gpsimd)
                    nc.scalar.dma_start(bo[:], bo_d[:])
                    nc.scalar.dma_start(wg[0][0:C, :], wg_d[0, 0:C])
                    nc.sync.dma_start(wg[0][C:FC, :], wg_d[0, C:FC])

                # ODE conv, both batch halves (block-diag weights)
                po = [psp.tile([FC, 512], F32, tag="ps", name=f"po{j}")
                      for j in range(2)]
                convj(po[0], wo, wcol, hbuf, 0, True, True)
                convj(po[1], wo, wcol, hbuf, 1, True, True)
                if t == 0:
                    nc.scalar.dma_start(bg[0][:], bg_d[0])
                    nc.sync.dma_start(bufa[0][C:FC, :, :], xs_d[0, 0:C])
                    nc.scalar.dma_start(bufa[1][0:C, :, :], xs_d[0, C:FC])
                    nc.sync.dma_start(wg[1][0:C, :], wg_d[1, 0:C])
                    nc.scalar.dma_start(wg[1][C:FC, :], wg_d[1, C:FC])
                    nc.scalar.dma_start(bg[1][:], bg_d[1])

                # tanh per bank; h_ode = h + t1 written straight into the
                # per-b gates rhs buffers (chunk c0 ready before c1)
                t1c = []
                for c in range(2):
                    tc_ = ew.tile([FC, 512], F32, tag="t1c", name=f"t1c{c}")
                    nc.scalar.activation(tc_[:], po[c][:], AF.Tanh,
                                         bias=bo[:, neg:neg + 1])
                    if dts[t] not in (1.0, -1.0):
                        nc.scalar.mul(tc_[:], tc_[:], float(dts[t]))
                    t1c.append(tc_)
                for c in range(2):
                    for b in range(BL):
                        ph = HD * b
                        nc.vector.tensor_add(
                            intrr(bufa[b], ph, HD, 16 * c, 16),
                            intrr(hbuf, ph, HD, 16 * c, 16),
                            r3c(t1c[c][ph:ph + HD, :]))

                # gates convs + per-bank sigmoid, rh, u; per-b tail prep
                # (u' = m*u, om = 1-u', f = om*h_ode) right after each b
                u = ew.tile([FC, NPIX], F32, tag="u")
                us = u
                if use_mask:
                    mt = ew.tile([FC, 1], F32, tag="mt")
                    for b in range(BL):
                        nc.sync.dma_start(mt[HD * b:HD * b + HD, :], msd[t, b])
                    us = ew.tile([FC, NPIX], F32, tag="u2")
                om = ew.tile([FC, NPIX], F32, tag="om")
                f = ew.tile([FC, NPIX], F32, tag="f")
                for b in range(BL):
                    ph, px = HD * b, HD * (1 - b)
                    pg = [psp.tile([FC, 512], F32, tag="ps", name=f"pg{b}{j}")
                          for j in range(2)]
                    convj(pg[0], wg[b], 0, bufa[b], 0, True, True)
                    convj(pg[1], wg[b], 0, bufa[b], 1, True, True)
                    for c in range(2):
                        gc = ew.tile([FC, 512], F32, tag="gtc",
                                     name=f"g{b}c{c}")
                        nc.scalar.activation(gc[:], pg[c][:], AF.Sigmoid,
                                             bias=bg[b][:])
                        nc.vector.tensor_mul(
                            intrr(rhbuf, ph, HD, 16 * c, 16),
                            r3c(gc[ph:ph + HD, :]),
                            intrr(bufa[b], ph, HD, 16 * c, 16))
                        nc.sync.dma_start(u[ph:ph + HD, 512 * c:512 * (c + 1)],
                                          gc[px:px + HD, :])
                    if use_mask:
                        nc.vector.tensor_single_scalar(
                            us[ph:ph + HD, :], u[ph:ph + HD, :],
                            mt[ph:ph + HD, :], mybir.AluOpType.mult)
                    nc.vector.tensor_scalar(om[ph:ph + HD, :],
                                            us[ph:ph + HD, :], -1.0, 1.0,
                                            mybir.AluOpType.mult,
                                            mybir.AluOpType.add)
                    nc.vector.tensor_mul(r3(f[ph:ph + HD, :]),
                                         r3(om[ph:ph + HD, :]),
                                         intr(bufa[b], ph, HD))


                if t == 0:
                    expand_diag(wch, wch_d[:], 9, nc.vector)
                    nc.scalar.dma_start(bc[:], bc_d[:])
                    nc.scalar.dma_start(wt1[:], wt1_d[:])
                    nc.scalar.dma_start(wt2[:], wt2_d[:])
                    nc.scalar.dma_start(bt1[:], bt1_d[:])
                    nc.scalar.dma_start(bt2[:], bt2_d[:])

                # canx j1: fills the sigmoid1/rh1 chain before canh
                convj(pc[1], wcx, 0, xbuf[t % 2], 1, True, False)

                if t + 1 < TD:
                    load_x(t + 1)

                # candidate conv, rh part (accumulates into pc banks)
                convj(pc[0], wch, 0, rhbuf, 0, False, True)
                convj(pc[1], wch, 0, rhbuf, 1, False, True)

                # tail: cand per bank, then h_next = f + u'*cand per chunk
                for c in range(2):
                    cc = ew.tile([FC, 512], F32, tag="candc", name=f"cc{c}")
                    nc.scalar.activation(cc[:], pc[c][:], AF.Tanh, bias=bc[:])
                    ec = ew.tile([FC, 512], F32, tag="ec", name=f"ec{c}")
                    nc.vector.tensor_mul(ec[:],
                                         us[:, 512 * c:512 * (c + 1)], cc[:])
                    nc.vector.tensor_add(
                        intrr(hbuf, 0, FC, 16 * c, 16),
                        r3c(f[:, 512 * c:512 * (c + 1)]),
                        r3c(ec[:]))

            # ---- transform_z0: conv1x1 -> ReLU -> conv1x1 --------------
            zc = []
            for j in range(2):
                ps1 = psp.tile([FC, 512], F32, tag="ps", name=f"ps1{j}")
                nc.tensor.matmul(ps1[:], wt1[:],
                                 hbuf[:, 1 + 16 * j:17 + 16 * j, 1:33],
                                 start=True, stop=True)
                z = ew.tile([FC, 512], MMD, tag="zc", name=f"zc{j}")
                nc.scalar.activation(z[:], ps1[:], AF.Relu, bias=bt1[:])
                zc.append(z)
            for b in range(BL):
                ph = HD * b
                for j in range(2):
                    ps2 = psp.tile([FC, 512], F32, tag="ps", name=f"ps2{b}{j}")
                    nc.tensor.matmul(ps2[:], wt2[ph:ph + HD, :],
                                     zc[j][ph:ph + HD, :],
                                     start=True, stop=True)
                    mso = ew.tile([FC, 512], F32, tag="mso", name=f"mso{b}{j}")
                    nc.scalar.activation(mso[0:HD, :], ps2[0:HD, :],
                                         AF.Identity, bias=bt2[0:HD, :])
                    nc.scalar.activation(mso[HD:FC, :], ps2[HD:FC, :],
                                         AF.Abs, bias=bt2[HD:FC, :])
                    nc.sync.dma_start(mean_d[b, :, 16 * j:16 * (j + 1), :],
                                      mso[0:HD, :])
                    nc.scalar.dma_start(std_d[b, :, 16 * j:16 * (j + 1), :],
                                        mso[HD:FC, :])

    nc.compile()
    return nc


def _conv2d_np(x, w, bias):
    Bn, Ci, Hn, Wn = x.shape
    O = w.shape[0]
    xp = np.pad(x, ((0, 0), (0, 0), (1, 1), (1, 1)))
    cols = np.empty((Bn, Ci, 9, Hn, Wn), np.float32)
    for k, (dy, dx) in enumerate(_offsets()):
        cols[:, :, k] = xp[:, :, dy:dy + Hn, dx:dx + Wn]
    out = np.matmul(w.reshape(O, Ci * 9)[None],
                    cols.reshape(Bn, Ci * 9, Hn * Wn))
    return (out + bias[None, :, None]).reshape(Bn, O, Hn, Wn)


def _sigmoid(v):
    return 1.0 / (1.0 + np.exp(-v))


def _host_step0(x, m, dt, w_gates, b_gates, w_can, b_can, b_ode):
    """Exact first recurrence step with h = 0 (so conv(h) == b_ode)."""
    Bn = x.shape[0]
    h_ode = np.broadcast_to((dt * np.tanh(b_ode)).astype(np.float32)
                            [None, :, None, None],
                            (Bn, HD, H, W)).astype(np.float32)
    comb = np.concatenate([x, h_ode], 1)
    gates = _sigmoid(_conv2d_np(comb, w_gates, b_gates))
    r, u = gates[:, :HD], gates[:, HD:]
    cand = np.tanh(_conv2d_np(np.concatenate([x, r * h_ode], 1),
                              w_can, b_can))
    h_new = (1.0 - u) * h_ode + u * cand
    mm = m[:, None, None, None]
    return (mm * h_new + (1.0 - mm) * h_ode).astype(np.float32)


def kernel(input_tensor, time_steps, mask, w_gates, b_gates, w_can, b_can,
           w_ode, b_ode, w_t1, b_t1, w_t2, b_t2):
    global last_result
    input_tensor = np.asarray(input_tensor, np.float32)
    time_steps = np.asarray(time_steps, np.float32)
    mask = np.asarray(mask, np.float32)
    w_gates = np.asarray(w_gates, np.float32)
    w_can = np.asarray(w_can, np.float32)
    w_ode = np.asarray(w_ode, np.float32)

    # host-side prep -------------------------------------------------
    # (T, C, B, H, W), time-reversed
    xs = np.transpose(input_tensor[:, ::-1], (1, 2, 0, 3, 4))
    ts_rev = time_steps[::-1].astype(np.float64)
    dts = np.concatenate([[-0.01], ts_rev[1:] - ts_rev[:-1]]).astype(np.float32)
    ms_all = mask[:, ::-1].T.astype(np.float32)      # (T, B)

    # first step on host (h starts at zero, and it is the only dt=-0.01 step)
    x_rev0 = np.ascontiguousarray(input_tensor[:, -1])       # (B, C, H, W)
    h1 = _host_step0(x_rev0, ms_all[0], float(dts[0]),
                     np.asarray(w_gates, np.float32),
                     np.asarray(b_gates, np.float32),
                     np.asarray(w_can, np.float32),
                     np.asarray(b_can, np.float32),
                     np.asarray(b_ode, np.float32))
    T0 = 1
    xs = xs[T0:]
    dts_dev = dts[T0:]
    ms_dev = ms_all[T0:]
    use_mask = not np.all(ms_dev == 1.0)

    FC = 2 * C
    swap = np.r_[C:FC, 0:C]
    ident = np.arange(FC)

    def lhsT9(w, in_perm, out_perm=None):
        o, i = w.shape[0], w.shape[1]
        out = np.empty((i, 9, o), np.float32)
        for k, (dy, dx) in enumerate(_offsets()):
            m = w[:, :, dy, dx].T[in_perm]
            if out_perm is not None:
                m = m[:, out_perm]
            out[:, k] = m
        return np.ascontiguousarray(out.reshape(i, 9 * o))

    def bdiag9(w):  # (64,64,3,3) -> block-diag (128, 9*128)
        out = np.zeros((FC, 9, FC), np.float32)
        for k, (dy, dx) in enumerate(_offsets()):
            m = w[:, :, dy, dx].T
            out[0:C, k, 0:C] = m
            out[C:FC, k, C:FC] = m
        return np.ascontiguousarray(out.reshape(FC, 9 * FC))

    wg_h = np.stack([lhsT9(w_gates, swap),
                     lhsT9(w_gates, ident, out_perm=swap)])
    def dense9(w):  # (64,64,3,3) -> (64, 9*64) lhsT blocks
        out = np.empty((C, 9, C), np.float32)
        for k, (dy, dx) in enumerate(_offsets()):
            out[:, k] = w[:, :, dy, dx].T
        return np.ascontiguousarray(out.reshape(C, 9 * C))

    wcx_h = dense9(w_can[:, 0:C])
    wch_h = dense9(w_can[:, C:FC])
    need_plain = any(float(dt) != -1.0 for dt in dts[1:])
    need_neg = any(float(dt) == -1.0 for dt in dts[1:])
    wo_parts = []
    if need_plain:
        wo_parts.append(dense9(w_ode))
    if need_neg:
        wo_parts.append(dense9(-w_ode))
    wo_h = np.concatenate(wo_parts, axis=1)
    wt1m = np.asarray(w_t1, np.float32)[:, :, 0, 0].T
    wt1_h = np.zeros((FC, FC), np.float32)
    wt1_h[0:C, 0:C] = wt1m
    wt1_h[C:FC, C:FC] = wt1m
    wt2_h = np.concatenate([np.asarray(w_t2, np.float32)[:, :, 0, 0].T] * 2, 0)

    bgn = np.asarray(b_gates, np.float32)
    bon = np.asarray(b_ode, np.float32)
    dup = lambda v: np.concatenate([v, v]).reshape(-1, 1)

    common = {
        "wg": wg_h, "wcx": wcx_h, "wch": wch_h, "wo": wo_h,
        "wt1": wt1_h, "wt2": wt2_h,
        "bg": np.stack([bgn.reshape(-1, 1), bgn[swap].reshape(-1, 1)]),
        "bc": dup(np.asarray(b_can, np.float32)),
        "bo": np.ascontiguousarray(np.concatenate([dup(bon), dup(-bon)], axis=1)),
        "bt1": dup(np.asarray(b_t1, np.float32)),
        "bt2": np.asarray(b_t2, np.float32).reshape(FC, 1),
    }

    in_maps = []
    for core in range(NCORES):
        bsl = slice(core * BL, (core + 1) * BL)
        m = dict(common)
        xp = np.zeros((T - T0, FC, P, P), np.float32)
        xp[:, 0:C, 1:33, 1:33] = xs[:, :, core * BL]
        xp[:, C:FC, 1:33, 1:33] = xs[:, :, core * BL + 1]
        m["xs"] = xp
        hp = np.zeros((FC, P, P), np.float32)
        hp[:, 1:33, 1:33] = h1[bsl].reshape(BL * HD, H, W)
        m["h0"] = hp
        if use_mask:
            mcore = ms_dev[:, bsl]
            m["ms"] = np.ascontiguousarray(
                np.broadcast_to(mcore[:, :, None, None],
                                (T - T0, BL, HD, 1))).astype(np.float32)
        in_maps.append(m)

    nc = _build(dts_dev, use_mask, T0)

    trace = bool(int(os.environ.get("KERNEL_TRACE", "0")))
    res = bass_utils.run_bass_kernel_spmd(
        nc, in_maps, core_ids=list(range(NCORES)), trace=trace)
    last_result = res

    mean = np.empty((B, HD, H, W), np.float32)
    std = np.empty((B, HD, H, W), np.float32)
    for core in range(NCORES):
        mean[core * BL:(core + 1) * BL] = res.results[core]["mean"]
        std[core * BL:(core + 1) * BL] = res.results[core]["std"]
    return mean, std


# revision 41
# speedup vs baseline: 1.0021x; 1.0021x over previous
"""Trainium2 Bass kernel for the Encoder-z0 ODE-ConvGRU problem.

Data-parallel over batch: 16 batch elements / 8 NeuronCores = 2 per core.
Per core, a 16-step backwards ConvGRU recurrence with an Euler ODE step,
followed by a 1x1-conv transform producing (mean_z0, std_z0).

Conv3x3 (SAME) is computed as 9 shifted matmuls accumulating in PSUM:
feature maps live in SBUF as zero-padded (34x34) images with channels on
partitions; offset (dy,dx) contributes lhsT[k].T @ shifted_view(rhs).

The two local batch elements are laid out on opposite partition halves
(b=0: 0-63, b=1: 64-127).  All M=64 convolutions (ODE, candidate halves,
first 1x1) are merged across the two batch elements into single full-array
K=128 x M=128 matmuls with block-diagonal weights, halving their PE time.
The candidate conv splits into an x-part (independent of the recurrent
state) whose two 9-matmul halves are scheduled to plug the recurrence's
two serial stalls, and an rh-part accumulating into the same PSUM banks.

Each conv output-row half gets its own single-bank PSUM tile and its own
dense SBUF activation tile, so Tile's (tile-granular) dependency tracking
yields precise chains: activations start as soon as their bank's
accumulation group stops, and the h_next/h_ode elementwise chains expose
only ~1-2us per step, hidden behind filler matmuls.

dt = -1 steps fold the Euler scale into negated ODE weights (tanh is odd).
Matmuls run in float32r (full-rate PE streaming, ~1e-3 accuracy);
elementwise runs in fp32.
"""

import os

import numpy as np

import concourse.bass as bass
import concourse.tile as tile
from concourse import bacc, mybir
from concourse import bass_utils

B, T, C, H, W = 16, 16, 64, 32, 32
HD = 64
NCORES = 8
BL = B // NCORES          # batch elements per core
P = H + 2                 # padded image edge (34)
NPIX = H * W              # 1024
MMD = mybir.dt.float32r   # matmul dtype
F32 = mybir.dt.float32

last_result = None


def _offsets():
    return [(dy, dx) for dy in range(3) for dx in range(3)]


def _build(dts, use_mask, t0):
    nc = bacc.Bacc("TRN2", target_bir_lowering=False, debug=False,
                   num_devices=NCORES)

    FC = 2 * C  # 128
    TD = T - t0  # device steps
    xs_d = nc.dram_tensor("xs", [TD, FC, P, P], MMD, kind="ExternalInput").ap()
    h0_d = nc.dram_tensor("h0", [FC, P, P], MMD, kind="ExternalInput").ap()
    wg_d = nc.dram_tensor("wg", [BL, FC, 9 * FC], MMD, kind="ExternalInput").ap()
    need_plain = any(float(dt) != -1.0 for dt in dts)
    need_neg = any(float(dt) == -1.0 for dt in dts)
    nv = int(need_plain) + int(need_neg)
    wcx_d = nc.dram_tensor("wcx", [C, 9 * C], MMD, kind="ExternalInput").ap()
    wch_d = nc.dram_tensor("wch", [C, 9 * C], MMD, kind="ExternalInput").ap()
    wo_d = nc.dram_tensor("wo", [C, nv * 9 * C], MMD, kind="ExternalInput").ap()
    wt1_d = nc.dram_tensor("wt1", [FC, FC], MMD, kind="ExternalInput").ap()
    wt2_d = nc.dram_tensor("wt2", [FC, FC], MMD, kind="ExternalInput").ap()
    bg_d = nc.dram_tensor("bg", [BL, FC, 1], F32, kind="ExternalInput").ap()
    bc_d = nc.dram_tensor("bc", [FC, 1], F32, kind="ExternalInput").ap()
    bo_d = nc.dram_tensor("bo", [FC, 2], F32, kind="ExternalInput").ap()
    bt1_d = nc.dram_tensor("bt1", [FC, 1], F32, kind="ExternalInput").ap()
    bt2_d = nc.dram_tensor("bt2", [FC, 1], F32, kind="ExternalInput").ap()
    if use_mask:
        msd = nc.dram_tensor("ms", [TD, BL, HD, 1], F32, kind="ExternalInput").ap()
    mean_d = nc.dram_tensor("mean", [BL, HD, H, W], F32, kind="ExternalOutput").ap()
    std_d = nc.dram_tensor("std", [BL, HD, H, W], F32, kind="ExternalOutput").ap()

    AF = mybir.ActivationFunctionType
    offs = _offsets()

    with tile.TileContext(nc) as tc:
        with (
            tc.tile_pool(name="persist", bufs=1) as pp,
            tc.tile_pool(name="ew", bufs=3) as ew,
            tc.tile_pool(name="psum", bufs=8, space="PSUM") as psp,
        ):
            # ---- persistent state ----
            hbuf = pp.tile([FC, P, P], MMD, name="hbuf")    # h: b0 low, b1 high
            xbuf = [pp.tile([FC, P, P], MMD, name=f"xbuf{i}")  # double-buffered
                    for i in range(2)]
            rhbuf = pp.tile([FC, P, P], MMD, name="rhbuf")  # r*h_ode per half
            bufa = [pp.tile([FC, P, P], MMD, name=f"bufa{b}") for b in range(BL)]
            wg = [pp.tile([FC, 9 * FC], MMD, name=f"wg{b}") for b in range(BL)]
            wcx = pp.tile([FC, 9 * FC], MMD, name="wcx")
            wch = pp.tile([FC, 9 * FC], MMD, name="wch")
            wo = pp.tile([FC, nv * 9 * FC], MMD, name="wo")
            wt1 = pp.tile([FC, FC], MMD, name="wt1")
            wt2 = pp.tile([FC, FC], MMD, name="wt2")
            bg = [pp.tile([FC, 1], F32, name=f"bg{b}") for b in range(BL)]
            bc = pp.tile([FC, 1], F32, name="bc")
            bo = pp.tile([FC, 2], F32, name="bo")           # [plain, negated]
            bt1 = pp.tile([FC, 1], F32, name="bt1")
            bt2 = pp.tile([FC, 1], F32, name="bt2")

            # zero only the pad borders (interiors are written before use)
            engs = [nc.vector, nc.gpsimd]
            for i, buf in enumerate([rhbuf, bufa[0], bufa[1]]):
                e = engs[i % 2]
                e.memzero(buf[:, 0, :])
                e.memzero(buf[:, 33, :])
                e.memzero(buf[:, 1:33, 0:1])
                e.memzero(buf[:, 1:33, 33:34])
            def expand_diag(wtile, src, nq, eng):
                """Fill the two diagonal 64x64 blocks of each 128-col chunk
                from a dense (64, nq*64) DRAM tensor (upper half zeroed by
                memzero beforehand)."""
                eng.memzero(wtile[:])
                srcv = src.rearrange("p (q m) -> p q m", m=C)
                for hh in range(2):
                    dst = wtile[C * hh:C * hh + C, :].rearrange(
                        "p (q m) -> p q m", m=FC)[:, :, C * hh:C * hh + C]
                    (nc.sync if hh == 0 else nc.scalar).dma_start(dst, srcv)

            # staged init: the first conv blocks wait only on their own
            # transfers; big loads are split across queues for bandwidth
            for q in range(4):
                eng = nc.sync if q % 2 == 0 else nc.scalar
                eng.dma_start(xbuf[0][32 * q:32 * (q + 1), :, :],
                              xs_d[0, 32 * q:32 * (q + 1)])
            expand_diag(wcx, wcx_d[:], 9, nc.vector)

            def convj(psum_t, wtile, wcol0, rhs_buf, j, first, last):
                """Conv matmuls for output-row half j into a 1-bank tile."""
                r0 = 16 * j
                for k, (dy, dx) in enumerate(offs):
                    nc.tensor.matmul(
                        psum_t[:],
                        wtile[:, wcol0 + FC * k:wcol0 + FC * (k + 1)],
                        rhs_buf[:, dy + r0:dy + r0 + 16, dx:dx + 32],
                        start=(first and k == 0), stop=(last and k == 8),
                        skip_group_check=True,
                    )

            def intr(buf, p0, pn):
                return buf[p0:p0 + pn, 1:33, 1:33]

            def intrr(buf, p0, pn, r0, rn):
                return buf[p0:p0 + pn, 1 + r0:1 + r0 + rn, 1:33]

            def r3c(ap):  # dense 512-col chunk -> (p, 16, 32)
                return ap.rearrange("p (y x) -> p y x", y=16, x=W)

            def r3(ap):
                return ap.rearrange("p (y x) -> p y x", y=H, x=W)

            def load_x(t):
                nc.sync.dma_start(xbuf[t % 2][:], xs_d[t])
                nc.sync.dma_start(bufa[0][C:FC, :, :], xs_d[t, 0:C])
                nc.sync.dma_start(bufa[1][0:C, :, :], xs_d[t, C:FC])

            load_x(0)

            for t in range(TD):
                if dts[t] == -1.0:
                    wcol = 9 * FC * int(need_plain)
                    neg = 1
                else:
                    wcol = 0
                    neg = 0

                # canx j0: fills the previous step's h_next tail
                pc = [psp.tile([FC, 512], F32, tag="ps", name=f"pc{j}")
                      for j in range(2)]
                convj(pc[0], wcx, 0, xbuf[t % 2], 0, True, False)

                if t == 0:
                    for q in range(4):
                        eng = nc.sync if q % 2 == 0 else nc.scalar
                        eng.dma_start(hbuf[32 * q:32 * (q + 1), :, :],
                                      h0_d[32 * q:32 * (q + 1)])
                    expand_diag(wo, wo_d[:], nv * 9, nc.gpsimd)
                    nc.scalar.dma_start(bo[:], bo_d[:])

                # ODE conv, both batch halves (block-diag weights)
                po = [psp.tile([FC, 512], F32, tag="ps", name=f"po{j}")
                      for j in range(2)]
                convj(po[0], wo, wcol, hbuf, 0, True, True)
                convj(po[1], wo, wcol, hbuf, 1, True, True)
                if t == 0:
                    nc.scalar.dma_start(wg[0][0:C, :], wg_d[0, 0:C])
                    nc.sync.dma_start(wg[0][C:FC, :], wg_d[0, C:FC])
                    nc.scalar.dma_start(bg[0][:], bg_d[0])
                    nc.sync.dma_start(bufa[0][C:FC, :, :], xs_d[0, 0:C])
                    nc.scalar.dma_start(bufa[1][0:C, :, :], xs_d[0, C:FC])
                    nc.sync.dma_start(wg[1][0:C, :], wg_d[1, 0:C])
                    nc.scalar.dma_start(wg[1][C:FC, :], wg_d[1, C:FC])
                    nc.scalar.dma_start(bg[1][:], bg_d[1])

                # tanh per bank; h_ode = h + t1 written straight into the
                # per-b gates rhs buffers (chunk c0 ready before c1)
                t1c = []
                for c in range(2):
                    tc_ = ew.tile([FC, 512], F32, tag="t1c", name=f"t1c{c}")
                    nc.scalar.activation(tc_[:], po[c][:], AF.Tanh,
                                         bias=bo[:, neg:neg + 1])
                    if dts[t] not in (1.0, -1.0):
                        nc.scalar.mul(tc_[:], tc_[:], float(dts[t]))
                    t1c.append(tc_)
                for c in range(2):
                    for b in range(BL):
                        ph = HD * b
                        nc.vector.tensor_add(
                            intrr(bufa[b], ph, HD, 16 * c, 16),
                            intrr(hbuf, ph, HD, 16 * c, 16),
                            r3c(t1c[c][ph:ph + HD, :]))

                # gates convs + per-bank sigmoid, rh, u; per-b tail prep
                # (u' = m*u, om = 1-u', f = om*h_ode) right after each b
                u = ew.tile([FC, NPIX], F32, tag="u")
                us = u
                if use_mask:
                    mt = ew.tile([FC, 1], F32, tag="mt")
                    for b in range(BL):
                        nc.sync.dma_start(mt[HD * b:HD * b + HD, :], msd[t, b])
                    us = ew.tile([FC, NPIX], F32, tag="u2")
                om = ew.tile([FC, NPIX], F32, tag="om")
                f = ew.tile([FC, NPIX], F32, tag="f")
                for b in range(BL):
                    ph, px = HD * b, HD * (1 - b)
                    pg = [psp.tile([FC, 512], F32, tag="ps", name=f"pg{b}{j}")
                          for j in range(2)]
                    convj(pg[0], wg[b], 0, bufa[b], 0, True, True)
                    convj(pg[1], wg[b], 0, bufa[b], 1, True, True)
                    for c in range(2):
                        gc = ew.tile([FC, 512], F32, tag="gtc",
                                     name=f"g{b}c{c}")
                        nc.scalar.activation(gc[:], pg[c][:], AF.Sigmoid,
                                             bias=bg[b][:])
                        nc.vector.tensor_mul(
                            intrr(rhbuf, ph, HD, 16 * c, 16),
                            r3c(gc[ph:ph + HD, :]),
                            intrr(bufa[b], ph, HD, 16 * c, 16))
                        nc.sync.dma_start(u[ph:ph + HD, 512 * c:512 * (c + 1)],
                                          gc[px:px + HD, :])
                    if use_mask:
                        nc.vector.tensor_single_scalar(
                            us[ph:ph + HD, :], u[ph:ph + HD, :],
                            mt[ph:ph + HD, :], mybir.AluOpType.mult)
                    nc.vector.tensor_scalar(om[ph:ph + HD, :],
                                            us[ph:ph + HD, :], -1.0, 1.0,
                                            mybir.AluOpType.mult,
                                            mybir.AluOpType.add)
                    nc.vector.tensor_mul(r3(f[ph:ph + HD, :]),
                                         r3(om[ph:ph + HD, :]),
                                         intr(bufa[b], ph, HD))


                if t == 0:
                    expand_diag(wch, wch_d[:], 9, nc.vector)
                    nc.scalar.dma_start(bc[:], bc_d[:])
                    nc.scalar.dma_start(wt1[:], wt1_d[:])
                    nc.scalar.dma_start(wt2[:], wt2_d[:])
                    nc.scalar.dma_start(bt1[:], bt1_d[:])
                    nc.scalar.dma_start(bt2[:], bt2_d[:])

                # canx j1: fills the sigmoid1/rh1 chain before canh
                convj(pc[1], wcx, 0, xbuf[t % 2], 1, True, False)

                if t + 1 < TD:
                    load_x(t + 1)

                # candidate conv, rh part (accumulates into pc banks)
                convj(pc[0], wch, 0, rhbuf, 0, False, True)
                convj(pc[1], wch, 0, rhbuf, 1, False, True)

                # tail: cand per bank, then h_next = f + u'*cand per chunk
                for c in range(2):
                    cc = ew.tile([FC, 512], F32, tag="candc", name=f"cc{c}")
                    nc.scalar.activation(cc[:], pc[c][:], AF.Tanh, bias=bc[:])
                    ec = ew.tile([FC, 512], F32, tag="ec", name=f"ec{c}")
                    nc.vector.tensor_mul(ec[:],
                                         us[:, 512 * c:512 * (c + 1)], cc[:])
                    nc.vector.tensor_add(
                        intrr(hbuf, 0, FC, 16 * c, 16),
                        r3c(f[:, 512 * c:512 * (c + 1)]),
                        r3c(ec[:]))

            # ---- transform_z0: conv1x1 -> ReLU -> conv1x1 --------------
            zc = []
            for j in range(2):
                ps1 = psp.tile([FC, 512], F32, tag="ps", name=f"ps1{j}")
                nc.tensor.matmul(ps1[:], wt1[:],
                                 hbuf[:, 1 + 16 * j:17 + 16 * j, 1:33],
                                 start=True, stop=True)
                z = ew.tile([FC, 512], MMD, tag="zc", name=f"zc{j}")
                nc.scalar.activation(z[:], ps1[:], AF.Relu, bias=bt1[:])
                zc.append(z)
            for b in range(BL):
                ph = HD * b
                for j in range(2):
                    ps2 = psp.tile([FC, 512], F32, tag="ps", name=f"ps2{b}{j}")
                    nc.tensor.matmul(ps2[:], wt2[ph:ph + HD, :],
                                     zc[j][ph:ph + HD, :],
                                     start=True, stop=True)
                    mso = ew.tile([FC, 512], F32, tag="mso", name=f"mso{b}{j}")
                    nc.scalar.activation(mso[0:HD, :], ps2[0:HD, :],
                                         AF.Identity, bias=bt2[0:HD, :])
                    nc.scalar.activation(mso[HD:FC, :], ps2[HD:FC, :],
                                         AF.Abs, bias=bt2[HD:FC, :])
                    nc.sync.dma_start(mean_d[b, :, 16 * j:16 * (j + 1), :],
                                      mso[0:HD, :])
                    nc.scalar.dma_start(std_d[b, :, 16 * j:16 * (j + 1), :],
                                        mso[HD:FC, :])

    nc.compile()
    return nc


def _conv2d_np(x, w, bias):
    Bn, Ci, Hn, Wn = x.shape
    O = w.shape[0]
    xp = np.pad(x, ((0, 0), (0, 0), (1, 1), (1, 1)))
    cols = np.empty((Bn, Ci, 9, Hn, Wn), np.float32)
    for k, (dy, dx) in enumerate(_offsets()):
        cols[:, :, k] = xp[:, :, dy:dy + Hn, dx:dx + Wn]
    out = np.matmul(w.reshape(O, Ci * 9)[None],
                    cols.reshape(Bn, Ci * 9, Hn * Wn))
    return (out + bias[None, :, None]).reshape(Bn, O, Hn, Wn)


def _sigmoid(v):
    return 1.0 / (1.0 + np.exp(-v))


def _host_step0(x, m, dt, w_gates, b_gates, w_can, b_can, b_ode):
    """Exact first recurrence step with h = 0 (so conv(h) == b_ode)."""
    Bn = x.shape[0]
    h_ode = np.broadcast_to((dt * np.tanh(b_ode)).astype(np.float32)
                            [None, :, None, None],
                            (Bn, HD, H, W)).astype(np.float32)
    comb = np.concatenate([x, h_ode], 1)
    gates = _sigmoid(_conv2d_np(comb, w_gates, b_gates))
    r, u = gates[:, :HD], gates[:, HD:]
    cand = np.tanh(_conv2d_np(np.concatenate([x, r * h_ode], 1),
                              w_can, b_can))
    h_new = (1.0 - u) * h_ode + u * cand
    mm = m[:, None, None, None]
    return (mm * h_new + (1.0 - mm) * h_ode).astype(np.float32)


def kernel(input_tensor, time_steps, mask, w_gates, b_gates, w_can, b_can,
           w_ode, b_ode, w_t1, b_t1, w_t2, b_t2):
    global last_result
    input_tensor = np.asarray(input_tensor, np.float32)
    time_steps = np.asarray(time_steps, np.float32)
    mask = np.asarray(mask, np.float32)
    w_gates = np.asarray(w_gates, np.float32)
    w_can = np.asarray(w_can, np.float32)
    w_ode = np.asarray(w_ode, np.float32)

    # host-side prep -------------------------------------------------
    # (T, C, B, H, W), time-reversed
    xs = np.transpose(input_tensor[:, ::-1], (1, 2, 0, 3, 4))
    ts_rev = time_steps[::-1].astype(np.float64)
    dts = np.concatenate([[-0.01], ts_rev[1:] - ts_rev[:-1]]).astype(np.float32)
    ms_all = mask[:, ::-1].T.astype(np.float32)      # (T, B)

    # first step on host (h starts at zero, and it is the only dt=-0.01 step)
    x_rev0 = np.ascontiguousarray(input_tensor[:, -1])       # (B, C, H, W)
    h1 = _host_step0(x_rev0, ms_all[0], float(dts[0]),
                     np.asarray(w_gates, np.float32),
                     np.asarray(b_gates, np.float32),
                     np.asarray(w_can, np.float32),
                     np.asarray(b_can, np.float32),
                     np.asarray(b_ode, np.float32))
    T0 = 1
    xs = xs[T0:]
    dts_dev = dts[T0:]
    ms_dev = ms_all[T0:]
    use_mask = not np.all(ms_dev == 1.0)

    FC = 2 * C
    swap = np.r_[C:FC, 0:C]
    ident = np.arange(FC)

    def lhsT9(w, in_perm, out_perm=None):
        o, i = w.shape[0], w.shape[1]
        out = np.empty((i, 9, o), np.float32)
        for k, (dy, dx) in enumerate(_offsets()):
            m = w[:, :, dy, dx].T[in_perm]
            if out_perm is not None:
                m = m[:, out_perm]
            out[:, k] = m
        return np.ascontiguousarray(out.reshape(i, 9 * o))

    def bdiag9(w):  # (64,64,3,3) -> block-diag (128, 9*128)
        out = np.zeros((FC, 9, FC), np.float32)
        for k, (dy, dx) in enumerate(_offsets()):
            m = w[:, :, dy, dx].T
            out[0:C, k, 0:C] = m
            out[C:FC, k, C:FC] = m
        return np.ascontiguousarray(out.reshape(FC, 9 * FC))

    wg_h = np.stack([lhsT9(w_gates, swap),
                     lhsT9(w_gates, ident, out_perm=swap)])
    def dense9(w):  # (64,64,3,3) -> (64, 9*64) lhsT blocks
        out = np.empty((C, 9, C), np.float32)
        for k, (dy, dx) in enumerate(_offsets()):
            out[:, k] = w[:, :, dy, dx].T
        return np.ascontiguousarray(out.reshape(C, 9 * C))

    wcx_h = dense9(w_can[:, 0:C])
    wch_h = dense9(w_can[:, C:FC])
    need_plain = any(float(dt) != -1.0 for dt in dts[1:])
    need_neg = any(float(dt) == -1.0 for dt in dts[1:])
    wo_parts = []
    if need_plain:
        wo_parts.append(dense9(w_ode))
    if need_neg:
        wo_parts.append(dense9(-w_ode))
    wo_h = np.concatenate(wo_parts, axis=1)
    wt1m = np.asarray(w_t1, np.float32)[:, :, 0, 0].T
    wt1_h = np.zeros((FC, FC), np.float32)
    wt1_h[0:C, 0:C] = wt1m
    wt1_h[C:FC, C:FC] = wt1m
    wt2_h = np.concatenate([np.asarray(w_t2, np.float32)[:, :, 0, 0].T] * 2, 0)

    bgn = np.asarray(b_gates, np.float32)
    bon = np.asarray(b_ode, np.float32)
    dup = lambda v: np.concatenate([v, v]).reshape(-1, 1)

    common = {
        "wg": wg_h, "wcx": wcx_h, "wch": wch_h, "wo": wo_h,
        "wt1": wt1_h, "wt2": wt2_h,
        "bg": np.stack([bgn.reshape(-1, 1), bgn[swap].reshape(-1, 1)]),
        "bc": dup(np.asarray(b_can, np.float32)),
        "bo": np.ascontiguousarray(np.concatenate([dup(bon), dup(-bon)], axis=1)),
        "bt1": dup(np.asarray(b_t1, np.float32)),
        "bt2": np.asarray(b_t2, np.float32).reshape(FC, 1),
    }

    in_maps = []
    for core in range(NCORES):
        bsl = slice(core * BL, (core + 1) * BL)
        m = dict(common)
        xp = np.zeros((T - T0, FC, P, P), np.float32)
        xp[:, 0:C, 1:33, 1:33] = xs[:, :, core * BL]
        xp[:, C:FC, 1:33, 1:33] = xs[:, :, core * BL + 1]
        m["xs"] = xp
        hp = np.zeros((FC, P, P), np.float32)
        hp[:, 1:33, 1:33] = h1[bsl].reshape(BL * HD, H, W)
        m["h0"] = hp
        if use_mask:
            mcore = ms_dev[:, bsl]
            m["ms"] = np.ascontiguousarray(
                np.broadcast_to(mcore[:, :, None, None],
                                (T - T0, BL, HD, 1))).astype(np.float32)
        in_maps.append(m)

    nc = _build(dts_dev, use_mask, T0)

    trace = bool(int(os.environ.get("KERNEL_TRACE", "0")))
    res = bass_utils.run_bass_kernel_spmd(
        nc, in_maps, core_ids=list(range(NCORES)), trace=trace)
    last_result = res

    mean = np.empty((B, HD, H, W), np.float32)
    std = np.empty((B, HD, H, W), np.float32)
    for core in range(NCORES):
        mean[core * BL:(core + 1) * BL] = res.results[core]["mean"]
        std[core * BL:(core + 1) * BL] = res.results[core]["std"]
    return mean, std
